# revision 27
# baseline (speedup 1.0000x reference)
"""Self-contained Trainium2 Bass kernel for nn_GCNResnet (batch-attention GCN).

Math (reference collapse):
  out[b,:] = sum_n c_n * softmax(X_n X_n^T)[b,:] @ (X_n @ W) + offset*(1_D @ W)
where X_n = x[:, n, :] ([B=4096, D=10]); c_n and offset fold BN(eval) +
adjacency + GCN + avgpool scalars. Per node the softmax normalizer is folded
into the PV matmul via a ones column:
  U_n = exp(X_n X_n^T) @ [c_n*(X_n@W) | 1]   -> out_n = U[:, :10] / U[:, 10]
(no max-subtraction needed: |scores| <= ~35 << 88, exp stays in fp32 range).

Sharding: row-slab parallel over 8 cores, 512 query rows per core, no
collectives. Steady state is exp-throughput-bound on the scalar engine
(6.3M elements/core at 153.6 G elem/s ~= 41 us); scores and PV matmuls
pipeline underneath on the PE.

v2 layout changes vs v1:
  - keys-split xt layout: scores are PE row-tiled 3-way; key chunk c lives
    ONLY at partition offset 32*(c%3), so keys are not replicated 3x
    (xt DMA 1.66MB -> 0.76MB per core). Queries (512 cols) are replicated
    into each of the 3 partition groups host-side.
  - xh in partition-major DRAM layout [128, N*KC*11]: loads as ONE DMA with
    2.1KB contiguous runs per partition. The v1 layout generated 12k+
    descriptors of 44B runs, which (not the scalar engine) was the real
    per-iteration bottleneck.
  - xh (PV lhsT: c_n*(X@W) | ones) and et (exp output, PV rhs) in bf16:
    full-rate matmul, half the DMA/SBUF of f32.
  - scalar-engine table warmup: a 1-element exp is issued before any real
    work so the ~2.7us exp_and_others table DMA overlaps the input loads.
  - PV_COLTILE exists (PE column tiling for the M=11 PV matmuls with
    host-side summation of the per-tile accumulators) but measured no
    better than 1 under the A/B noise floor, so it defaults to 1.

v3 pipeline changes (TimelineSim-guided: 62.4us -> 60.3us single-shot):
  - psum pool parity alternates per node ((g+n)%2): NG=11 is odd, so node
    n+1's first scores group lands in the OTHER buffer than node n's last,
    shrinking the node-boundary WAR stall.
  - U accumulator double-buffered (U_BUFS=2, uses the 8th PSUM bank).
  - xt DMAs spread over the gpsimd+SP DGE queues (per-DMA setup is ~1us;
    serializing all 9 on one queue delayed the first scores by ~2us); xh is
    issued right after node 0's xt so the first PV is never starved. Putting
    DMAs on the ACT queue delays the exp stream - don't.
  - no trailing all-engine barrier after semaphore cleanup: each engine
    stream just ends; repeat executions verified correct on HW.

v4 (sim 60.3 -> 59.4us): PV software-pipelined one group behind scores —
the PE issues group k+1's scores before group k's PV, so the exp stream is
never starved at group/node boundaries (boundary bubbles 0.94 -> 0.52us).
Per-node U drains are emitted when the lagged PV crosses a node boundary.

v5 (sim 59.3us): queries-first column order inside each xt tile block, and
node 0's loads split at a chunk boundary (QSPLIT = queries + 3 chunks) so
the head lands in ~half the transfer time and the first scores start
earlier. Tile dependency tracking is AP-range-based, so readers of the
head columns do not wait for the tail DMA. (PE clock-warmup dummies and
any DMA on the ACT queue both sim worse - do not retry.)

PSUM layout: two 3-bank scores buffers double-buffering each other (PE fills
one while ACT exps the other) + 1 bank for the U accumulator. The tiny final
divide+transpose (0.08% of FLOPs) happens on host during the gather step.
"""

import sys

if "/opt/trn_rl_repo" not in sys.path:
    sys.path.insert(0, "/opt/trn_rl_repo")

import numpy as np
import ml_dtypes

import concourse.bass as bass
import concourse.mybir as mybir
from concourse import tile
from concourse.bass_utils import run_bass_kernel_spmd
from concourse.vector_clock import ScopedClock

B, N, D = 4096, 3, 10
NCORES = 8
R = B // NCORES            # 512 query rows per core
KC = B // 128              # 32 key chunks of 128
NT = 3                     # PE row tiles
CPT = (KC + NT - 1) // NT  # key chunks per row tile (11; tile 2 has 10)
KW = CPT * 128             # key columns per tile block (1408)
TW = KW + R                # total columns per tile block (1920)
QSPLIT = R + 3 * 128       # node-0 head DMA: queries + first 3 chunks (896)
NG = CPT                   # score groups per node (11; last group width 2)
BN_EPS = 1e-5

USE_F32R_SCORES = True
ACT_WARMUP = True
PV_COLTILE = 2             # PE column tiles for PV (post-v5 A/B: 2 beats 1 ~2x)
ET_BUFS = 3
XT_BUFS = 2
U_BUFS = 2


def _patched_drain_and_barrier(self, tick_clock, wait_clock):
    # Walrus in this container rejects >1 sync-wait on a CTRL-class
    # instruction; absorb the tail-drain waits into SP nops, one wait each.
    nc = self.nc
    probe = nc.sync.nop()
    wait_clock.add_sem_waits(probe.ins, ScopedClock({None: tick_clock.global_clock}))
    si = probe.ins.sync_info
    waits = list(si.on_wait) if si is not None else []
    upds = list(si.on_update) if si is not None else []
    probe.ins.sync_info = mybir.SyncInfo(on_wait=waits[:1], on_update=upds)
    for w in waits[1:]:
        n = nc.sync.nop()
        n.ins.sync_info = mybir.SyncInfo(on_wait=[w], on_update=[])
    nc.sync.drain()
    nc.all_engine_barrier()
    assert self.sems is not None
    popped = nc._tile_sem_poison_stack.pop()
    assert popped is self._sem_poison
    nc.clear_and_free_semaphores(list(self.sems.allocated().values()))
    # no trailing barrier: each engine stream simply ends after its cleanup
    # ops; the NEFF completes when all queues drain.


tile.TileContext._drain_and_barrier = _patched_drain_and_barrier

_MAX_WAITS = 1
_waitsplit_ctr = [0]


def _split_sync_waits(nc):
    """Walrus here allows very few sync-waits per instruction. Move excess
    waits onto same-engine no-ops placed immediately before the instruction
    (engine streams are in-order, so semantics are preserved)."""
    for f in nc.m.functions:
        for bb in f.blocks:
            new = []
            changed = False
            for inst in bb.instructions:
                si = inst.sync_info
                waits = list(si.on_wait) if si is not None else []
                if len(waits) > _MAX_WAITS:
                    changed = True
                    for w in waits[:-_MAX_WAITS]:
                        _waitsplit_ctr[0] += 1
                        nop = mybir.InstNoOp(
                            name=f"I-waitsplit-{_waitsplit_ctr[0]}", ins=[], outs=[]
                        )
                        nop.engine = inst.engine
                        nop.sync_info = mybir.SyncInfo(on_wait=[w], on_update=[])
                        new.append(nop)
                    inst.sync_info = mybir.SyncInfo(
                        on_wait=waits[-_MAX_WAITS:], on_update=list(si.on_update)
                    )
                new.append(inst)
            if changed:
                bb.instructions = new


def _groups():
    """Score groups: group g covers chunks 3g+i for row tiles i with
    3g+i < KC. All groups have width 3 except the last (width 2)."""
    return [[3 * g + i for i in range(NT) if 3 * g + i < KC] for g in range(NG)]


def build_nc(rep: int = 1, rep_marker: bool = False, mode: str = "full") -> bass.Bass:
    """One-core SPMD program. mode: "full", "loads", "compute", "nopv"
    (timing decomposition)."""
    f32 = mybir.dt.float32
    bf16 = mybir.dt.bfloat16
    sdt = mybir.dt.float32r if USE_F32R_SCORES else f32
    nc = bass.Bass()

    # keys-split xt: [node, row-tile, D, KW keys + R queries]. Declared f32r
    # directly (same bits as f32, np side sees float32) so the loads need no
    # casting DMA and can ride any DGE queue.
    xtq = nc.declare_dram_parameter("xtq", [N, NT, D, TW], sdt, isOutput=False)
    xh = nc.declare_dram_parameter("xh", [128, N * KC * (D + 1)], bf16, isOutput=False)
    uout = nc.declare_dram_parameter(
        "uout", [(D + 1) * PV_COLTILE + 2, 512 * N], f32, isOutput=True
    )

    groups = _groups()

    with tile.TileContext(nc) as tc:
        with (
            tc.tile_pool(name="xtp", bufs=XT_BUFS) as xtp,
            tc.tile_pool(name="xhp", bufs=XT_BUFS) as xhp,
            tc.tile_pool(name="etp", bufs=ET_BUFS) as etp,
            tc.tile_pool(name="mrk", bufs=1) as mrkp,
            tc.tile_pool(name="pssA", bufs=1, space="PSUM") as pssA,
            tc.tile_pool(name="pssB", bufs=1, space="PSUM") as pssB,
            tc.tile_pool(name="psu", bufs=U_BUFS, space="PSUM") as psu,
        ):
            if ACT_WARMUP:
                wtile = mrkp.tile([1, 2], f32, tag="warm")
                nc.vector.memset(wtile[:], 0.0)
                nc.scalar.activation(
                    wtile[:], wtile[:], mybir.ActivationFunctionType.Exp
                )
            xt_sb = xh_sb = None
            for rep_i in range(rep):
                if mode != "compute" or rep_i == 0:
                    # ---- input loads ----
                    # xt: per (node, row-tile) one casting DMA into partitions
                    # 32i..32i+9 (keys for chunks c%3==i, then the query slab).
                    # spread the xt DMAs over two DGE queues so their ~1us
                    # setups overlap; node 0's three tiles land first. The
                    # ACT queue stays clear for the exp stream. (A single
                    # per-node DMA via a 2-level partition AP compiles but
                    # sims worse: transfer time is per-partition bytes, and
                    # the split keeps cross-queue overlap.)
                    xt_engs = [nc.gpsimd, nc.sync, nc.gpsimd]
                    xt_sb = [
                        xtp.tile([128, TW], sdt, tag=f"xt{n}", name=f"xt{n}")
                        for n in range(N)
                    ]
                    xh_sb = xhp.tile([128, N * KC * (D + 1)], bf16)
                    for n in range(N):
                        for i in range(NT):
                            if n == 0:
                                # split node 0 at a chunk boundary: the head
                                # (queries + 3 chunks, ~3.5KB/partition) lands
                                # in half the transfer time of the full tile
                                xt_engs[i].dma_start(
                                    xt_sb[n][32 * i : 32 * i + D, 0:QSPLIT],
                                    xtq[n, i, :, 0:QSPLIT],
                                )
                            else:
                                xt_engs[i].dma_start(
                                    xt_sb[n][32 * i : 32 * i + D, :], xtq[n, i]
                                )
                        if n == 0:
                            nc.sync.dma_start(xh_sb[:], xh[:, :])
                            for i in range(NT):
                                xt_engs[i].dma_start(
                                    xt_sb[n][32 * i : 32 * i + D, QSPLIT:TW],
                                    xtq[n, i, :, QSPLIT:TW],
                                )
                if mode == "loads":
                    continue

                flat = [(n, g, chunks) for n in range(N)
                        for g, chunks in enumerate(groups)]
                u_tiles = {}
                pend = None  # (n, chunks, et) whose PV is not yet issued

                def emit_pv(n, chunks, et):
                    for i, ck in enumerate(chunks):
                        j = ck % PV_COLTILE
                        nc.tensor.matmul(
                            u_tiles[n][32 * j : 32 * j + D + 1, :],
                            lhsT=xh_sb[
                                :,
                                (n * KC + ck) * (D + 1) : (n * KC + ck + 1)
                                * (D + 1),
                            ],
                            rhs=et[:, 512 * i : 512 * (i + 1)],
                            start=(ck < PV_COLTILE),
                            stop=(ck >= KC - PV_COLTILE),
                            tile_position=(0, 32 * j),
                        )

                def emit_drain(n):
                    u_sb = etp.tile(
                        [128, 512], f32, tag="usb", bufs=2, name="u_sb"
                    )
                    for j in range(PV_COLTILE):
                        nc.vector.tensor_copy(
                            u_sb[32 * j : 32 * j + D + 1, :],
                            u_tiles[n][32 * j : 32 * j + D + 1, :],
                        )
                        nc.sync.dma_start(
                            uout[
                                (D + 1) * j : (D + 1) * (j + 1),
                                512 * n : 512 * (n + 1),
                            ],
                            u_sb[32 * j : 32 * j + D + 1, :],
                        )

                for n, g, chunks in flat:
                    if g == 0:
                        u_tiles[n] = psu.tile([128, 512], f32, tag="u", name="u_ps")
                    w = len(chunks)
                    # NG is odd: parity offset by node so consecutive groups
                    # across a node boundary use different psum pools
                    pool = pssB if (g + n) % 2 else pssA
                    ps = pool.tile(
                        [128, 512 * NT],
                        f32,
                        tag=f"s{(g + n) % 2}",
                        name=f"s{(g + n) % 2}",
                    )
                    for i in range(w):
                        nc.tensor.matmul(
                            ps[:, 512 * i : 512 * (i + 1)],
                            lhsT=xt_sb[n][
                                32 * i : 32 * i + D,
                                R + 128 * g : R + 128 * (g + 1),
                            ],
                            rhs=xt_sb[n][32 * i : 32 * i + D, 0:R],
                            tile_position=(32 * i, 0),
                        )
                    et = etp.tile([128, 512 * NT], bf16, tag="et")
                    nc.scalar.activation(
                        et[:, : 512 * w],
                        ps[:, : 512 * w],
                        mybir.ActivationFunctionType.Exp,
                    )
                    if mode == "nopv":
                        continue
                    # PV lags one group: the PE issues the NEXT group's
                    # scores before the PREVIOUS group's PV, so the exp
                    # stream is never starved at group/node boundaries
                    if pend is not None:
                        pn, pchunks, pet = pend
                        emit_pv(pn, pchunks, pet)
                        if pn != n:
                            emit_drain(pn)
                    pend = (n, chunks, et)
                if mode != "nopv" and pend is not None:
                    pn, pchunks, pet = pend
                    emit_pv(pn, pchunks, pet)
                    emit_drain(pn)
                if rep_marker and mode != "nopv":
                    mark = mrkp.tile([1, 4], f32, tag="mark")
                    nc.vector.memset(mark[:], float(rep_i))
                    nc.sync.dma_start(
                        uout[(D + 1) * PV_COLTILE : (D + 1) * PV_COLTILE + 1, 0:4],
                        mark[:],
                    )
    _split_sync_waits(nc)
    return nc


def _host_prep(x, A, gc_weight, bn_gamma, bn_beta, bn_mean, bn_var):
    x = np.asarray(x, np.float32)
    A = np.asarray(A, np.float32)
    W = np.asarray(gc_weight, np.float32)
    scale = np.asarray(bn_gamma, np.float32) / np.sqrt(
        np.asarray(bn_var, np.float32) + BN_EPS
    )
    d_half = 0.5 * np.eye(N, dtype=np.float32)
    a0 = np.ones((N, N), np.float32) - np.eye(N, dtype=np.float32)
    adj = d_half @ (a0 + A) @ d_half
    wk = 0.5 * (adj[0] + adj[1])                      # [N]
    cn = (wk * scale).astype(np.float32)              # [N]
    offset = float(
        np.sum(wk * (np.asarray(bn_beta, np.float32)
                     - np.asarray(bn_mean, np.float32) * scale))
    )
    bias_vec = (offset * W.sum(axis=0)).astype(np.float32)  # [D]

    xt = x.transpose(1, 2, 0)                         # [N, D, B] (view)
    xh = np.empty((N, B, D + 1), np.float32)
    for n in range(N):
        xh[n, :, :D] = (x[:, n, :] @ W) * cn[n]
        xh[n, :, D] = 1.0
    # partition-major layout: dram[p, (n, c, d)] = xh[n, c, p, d] -> the
    # whole xh loads as ONE DMA with 2.1KB-contiguous runs per partition.
    xh16 = np.ascontiguousarray(
        xh.reshape(N, KC, 128, D + 1).transpose(2, 0, 1, 3).reshape(128, -1)
        .astype(ml_dtypes.bfloat16)
    )
    return xt, xh16, bias_vec


def _in_maps(xt, xh16):
    # keys-split layout: tile i holds key chunks c with c%3==i, then queries
    xkeys = np.zeros((N, NT, D, KW), np.float32)
    for i in range(NT):
        cks = [c for c in range(KC) if c % NT == i]
        blk = np.concatenate(
            [xt[:, :, 128 * c : 128 * (c + 1)] for c in cks], axis=2
        )  # [N, D, 128*len]
        xkeys[:, i, :, : blk.shape[2]] = blk
    maps = []
    for c in range(NCORES):
        q = xt[:, :, c * R : (c + 1) * R]             # [N, D, R]
        xtq = np.concatenate(
            [np.broadcast_to(q[:, None], (N, NT, D, R)), xkeys], axis=3
        ).astype(np.float32)                           # [N, NT, D, TW]
        maps.append({"xtq": np.ascontiguousarray(xtq), "xh": xh16})
    return maps


def _finish(uouts, bias_vec):
    """Host gather: normalize U (divide by the folded rowsum), transpose to
    [rows, D], sum nodes, concatenate core slabs, add the BN/adjacency bias."""
    out = np.empty((B, D), np.float32)
    for c in range(NCORES):
        u = uouts[c]                                   # [24, 512*N]
        acc = np.zeros((512, D), np.float32)
        for n in range(N):
            blk = u[:, 512 * n : 512 * (n + 1)]
            un = sum(
                blk[(D + 1) * j : (D + 1) * (j + 1)].astype(np.float64)
                for j in range(PV_COLTILE)
            )                                          # [11, 512]
            acc += (un[:D] / un[D]).T.astype(np.float32)
        out[c * R : (c + 1) * R] = acc
    return out + bias_vec[None, :]


def kernel(**inputs) -> np.ndarray:
    assert inputs["x"].shape == (B, N, D)
    xt, xh16, bias_vec = _host_prep(**inputs)
    nc = build_nc(rep=1)
    res = run_bass_kernel_spmd(nc, _in_maps(xt, xh16), list(range(NCORES)))
    return _finish(
        [res.results[c]["uout"] for c in range(NCORES)], bias_vec
    ).astype(np.float32)
